# revision 1
# baseline (speedup 1.0000x reference)
"""Trainium2 Bass kernel for nn_Decoder_68152541053662.

2-layer GAT (heads=1, self-loops) + sigmoid inner-product decoder.
  N=12000 nodes, E=384000 edges (+N self loops), feats 40 -> 50 -> 40,
  output sigmoid(z @ z.T)  [12000, 12000] f32.

Sharding: nodes row-partitioned across 8 cores (1500 dst rows each).
Each core aggregates only its own dst rows; full feature tables are
rebuilt on every core between layers with AllGather collectives.

Edge phase per layer (per core):
  - host packs incoming edges of each local dst into an ELL table
    (K=64 slots/dst, padded with a pointer to a dedicated pad row).
  - gathered rows come from an "augmented" feature table in DRAM:
    row = [h (F cols) | 1.0 | a_src . h | zero pad]  (64 f32 = 256 B),
    fetched with one dma_gather (8192 rows) per 128-dst block.
  - attention weights w = exp(leaky_relu(as[src] + ad[dst])) are computed
    on DVE/ACT in the gathered layout, then written into a mostly-zero
    "selection matrix" M with a fixed strided pattern; the segment-summed
    (and w-weighted) aggregation is a chain of 64 accumulating matmuls
    out[dst, :] = sum_t M[:, t, :].T @ T[:, t, :] which also produces the
    softmax denominator in the ones-column.  Skipping the segment-max
    subtraction is safe here (|e| <= ~10 for these magnitudes).
Decoder: S = sigmoid(z_own @ z_full.T) with float32r matmuls, row strip
written straight to DRAM.
"""

import numpy as np

try:
    import concourse.bass as bass
except ImportError:  # pragma: no cover
    import sys

    sys.path.insert(0, "/opt/trn_rl_repo")
    import concourse.bass as bass

import concourse.bacc as bacc
import concourse.tile as tile
from concourse import mybir
from concourse.bass_utils import run_bass_kernel_spmd

AF = mybir.ActivationFunctionType
ALU = mybir.AluOpType
F32 = mybir.dt.float32
F32R = mybir.dt.float32r
I16 = mybir.dt.int16

FULL_CFG = dict(N=12000, P=8, FIN=40, F1=50, F2=40, K=64, NEG=0.2, DEC_CHUNK=500)
DEBUG_EDGE_LVL = 3  # <3 truncates the edge phase (perf bisection only)
PAD_AS = -100.0  # "as" value of the pad row -> w ~ exp(-20) ~ 0


def derive(cfg):
    d = dict(cfg)
    d["NLOC"] = cfg["N"] // cfg["P"]
    d["NB"] = (d["NLOC"] + 127) // 128
    d["NLOCP"] = d["NB"] * 128
    d["G"] = 128 // cfg["K"]  # partition groups for M writes
    d["SPB"] = 128 * cfg["K"]  # gather slots per 128-dst block
    d["ICPB"] = d["SPB"] // 16  # idx columns per block
    d["NR"] = cfg["N"] + 1  # aug table rows (+1 pad row)
    d["NCH"] = cfg["N"] // cfg["DEC_CHUNK"]
    assert cfg["N"] % cfg["DEC_CHUNK"] == 0 and cfg["DEC_CHUNK"] <= 512
    assert cfg["N"] % cfg["P"] == 0 and 128 % cfg["K"] == 0
    return d


# --------------------------------------------------------------------------
# host-side preprocessing
# --------------------------------------------------------------------------


def build_ell(edge_index, cfg):
    """Per-core ELL tables in the wrapped int16 layout dma_gather wants.

    Slot order within a 128-dst block b: j = D*K + k (D = local dst in
    block, k = slot).  Flat slot index J = b*SPB + j; the int16 index for J
    sits at [16*g + J%16, J//16] for all replication groups g (the 8 gpsimd
    cores each read their own 16 partitions).
    """
    c = cfg
    N, P, K = c["N"], c["P"], c["K"]
    nloc, nb, nlocp = c["NLOC"], c["NB"], c["NLOCP"]
    src = np.asarray(edge_index[0], dtype=np.int64)
    dst = np.asarray(edge_index[1], dtype=np.int64)
    loops = np.arange(N, dtype=np.int64)
    src = np.concatenate([src, loops])
    dst = np.concatenate([dst, loops])

    order = np.argsort(dst, kind="stable")
    src, dst = src[order], dst[order]
    deg = np.bincount(dst, minlength=N)
    assert deg.max() <= K, f"max degree {deg.max()} > K={K}"
    starts = np.concatenate([[0], np.cumsum(deg)])

    # slots[d, k] = src of k-th incoming edge of node d (pad -> N)
    slots = np.full((N, K), N, dtype=np.int64)
    pos = np.arange(len(dst)) - starts[dst]
    slots[dst, pos] = src

    tabs = []
    for core in range(P):
        s = np.full((nlocp, K), N, dtype=np.int64)
        s[:nloc] = slots[core * nloc : (core + 1) * nloc]
        # flat order J = b*SPB + (D*K + k); D = local-in-block dst
        flat = s.reshape(nb, 128, K).reshape(-1)  # == J order
        wrapped = np.zeros((128, len(flat) // 16), dtype=np.int16)
        cols = flat.reshape(-1, 16).T.astype(np.int16)  # [16, J//16]
        for g in range(8):
            wrapped[16 * g : 16 * g + 16, :] = cols
        tabs.append(wrapped)
    return tabs


def make_inputs(x, edge_index, W1, a_src1, a_dst1, b1, W2, a_src2, a_dst2, b2, cfg):
    c = cfg
    N, P, FIN, F1, F2 = c["N"], c["P"], c["FIN"], c["F1"], c["F2"]
    nloc, nlocp = c["NLOC"], c["NLOCP"]
    x = np.asarray(x, dtype=np.float32)
    ell = build_ell(edge_index, c)

    def rep(v, f):
        r = np.zeros((128, f), dtype=np.float32)
        r[:] = np.asarray(v, dtype=np.float32)[None, :]
        return r

    pads = np.zeros((2, 64), dtype=np.float32)
    pads[0, F1] = 1.0
    pads[0, F1 + 1] = PAD_AS
    pads[1, F2] = 1.0
    pads[1, F2 + 1] = PAD_AS

    common = {
        "w1": np.asarray(W1, dtype=np.float32),
        "w2": np.asarray(W2, dtype=np.float32),
        "asrc1r": rep(a_src1, F1),
        "adst1r": rep(a_dst1, F1),
        "b1r": rep(b1, F1),
        "asrc2r": rep(a_src2, F2),
        "adst2r": rep(a_dst2, F2),
        "b2r": rep(b2, F2),
        "pads": pads,
        "eye": np.eye(128, dtype=np.float32),
    }
    maps = []
    for core in range(P):
        xt = np.zeros((FIN, nlocp), dtype=np.float32)
        xt[:, :nloc] = x[core * nloc : (core + 1) * nloc].T
        m = dict(common)
        m["xt"] = xt
        m["elli"] = ell[core]
        maps.append(m)
    return maps


# --------------------------------------------------------------------------
# device program
# --------------------------------------------------------------------------


def build_program(cfg, stop_after=None, repeat=1):
    c = cfg
    N, P, FIN, F1, F2, K, NEG = (
        c["N"], c["P"], c["FIN"], c["F1"], c["F2"], c["K"], c["NEG"],
    )
    nloc, nb, nlocp, G, SPB, ICPB, NR = (
        c["NLOC"], c["NB"], c["NLOCP"], c["G"], c["SPB"], c["ICPB"], c["NR"],
    )
    DCH, NCH = c["DEC_CHUNK"], c["NCH"]
    tail = nloc - 128 * (nb - 1)
    groups = [list(range(P))]

    nc = bacc.Bacc("TRN2", target_bir_lowering=False, debug=False, num_devices=P)

    # I/O
    xt_d = nc.dram_tensor("xt", [FIN, nlocp], F32, kind="ExternalInput")
    w1_d = nc.dram_tensor("w1", [FIN, F1], F32, kind="ExternalInput")
    w2_d = nc.dram_tensor("w2", [F1, F2], F32, kind="ExternalInput")
    asrc1r_d = nc.dram_tensor("asrc1r", [128, F1], F32, kind="ExternalInput")
    adst1r_d = nc.dram_tensor("adst1r", [128, F1], F32, kind="ExternalInput")
    b1r_d = nc.dram_tensor("b1r", [128, F1], F32, kind="ExternalInput")
    asrc2r_d = nc.dram_tensor("asrc2r", [128, F2], F32, kind="ExternalInput")
    adst2r_d = nc.dram_tensor("adst2r", [128, F2], F32, kind="ExternalInput")
    b2r_d = nc.dram_tensor("b2r", [128, F2], F32, kind="ExternalInput")
    pads_d = nc.dram_tensor("pads", [2, 64], F32, kind="ExternalInput")
    eye_d = nc.dram_tensor("eye", [128, 128], F32, kind="ExternalInput")
    elli_d = nc.dram_tensor("elli", [128, nb * ICPB], I16, kind="ExternalInput")
    out_d = nc.dram_tensor("out", [nloc, N], F32, kind="ExternalOutput")

    # internal DRAM
    haug1 = nc.dram_tensor("haug1", [NR, 64], F32)
    haug2 = nc.dram_tensor("haug2", [NR, 64], F32)
    ccin1 = nc.dram_tensor("ccin1", [nloc, 64], F32)
    ccout1 = nc.dram_tensor("ccout1", [N, 64], F32, addr_space="Shared")
    ccin2 = nc.dram_tensor("ccin2", [nloc, 64], F32)
    ccout2 = nc.dram_tensor("ccout2", [N, 64], F32, addr_space="Shared")
    adt1 = nc.dram_tensor("adt1", [nb, 128], F32)
    adt2 = nc.dram_tensor("adt2", [nb, 128], F32)
    ztin = nc.dram_tensor("ztin", [F2, nloc], F32)
    ztcc = nc.dram_tensor("ztcc", [P * F2, nloc], F32, addr_space="Shared")

    with tile.TileContext(nc) as tc:
      with tc.tile_pool(name="persist", bufs=1) as ppool:
        zt_own = ppool.tile([F2, nlocp], F32R)
        def _pipeline():
            with (
                tc.tile_pool(name="const", bufs=1) as cpool,
                tc.tile_pool(name="strips", bufs=1) as spool,
                tc.tile_pool(name="gat_small", bufs=3) as gpool,
                tc.tile_pool(name="gather", bufs=2) as tpool,
                tc.tile_pool(name="psum_small", bufs=2, space="PSUM") as pps,
                tc.tile_pool(name="psum_agg", bufs=4, space="PSUM") as pagg,
            ):
                # ---- constant loads -------------------------------------------------
                xt_sb = cpool.tile([FIN, nlocp], F32)
                nc.sync.dma_start(out=xt_sb[:, :], in_=xt_d[:, :])
                w1_sb = cpool.tile([FIN, F1], F32)
                nc.sync.dma_start(out=w1_sb[:, :], in_=w1_d[:, :])
                w2_sb = cpool.tile([F1, F2], F32)
                nc.sync.dma_start(out=w2_sb[:, :], in_=w2_d[:, :])
                asrc1_sb = cpool.tile([128, F1], F32)
                nc.sync.dma_start(out=asrc1_sb[:, :], in_=asrc1r_d[:, :])
                adst1_sb = cpool.tile([128, F1], F32)
                nc.sync.dma_start(out=adst1_sb[:, :], in_=adst1r_d[:, :])
                b1_sb = cpool.tile([128, F1], F32)
                nc.sync.dma_start(out=b1_sb[:, :], in_=b1r_d[:, :])
                asrc2_sb = cpool.tile([128, F2], F32)
                nc.sync.dma_start(out=asrc2_sb[:, :], in_=asrc2r_d[:, :])
                adst2_sb = cpool.tile([128, F2], F32)
                nc.sync.dma_start(out=adst2_sb[:, :], in_=adst2r_d[:, :])
                b2_sb = cpool.tile([128, F2], F32)
                nc.sync.dma_start(out=b2_sb[:, :], in_=b2r_d[:, :])
                eye_sb = cpool.tile([128, 128], F32)
                nc.sync.dma_start(out=eye_sb[:, :], in_=eye_d[:, :])
                elli_sb = cpool.tile([128, nb * ICPB], I16)
                nc.sync.dma_start(out=elli_sb[:, :], in_=elli_d[:, :])
                ones1_sb = cpool.tile([1, 128], F32)
                nc.vector.memset(ones1_sb[:, :], 1.0)

                # selection matrix M: [128, K, 128], zeroed once; the non-zero
                # pattern (G strided diagonals) is identical for every block.
                # two selection matrices, alternating per block, so block
                # b+1's w-scatter (DVE) never has to wait for block b's
                # 64-matmul chain to finish reading M
                m_sbs = []
                for _mi in range(2):
                    m_i = cpool.tile([128, K, 128], F32, name=f"m{_mi}", tag=f"m{_mi}")
                    nc.vector.memset(m_i[:, :, :], 0.0)
                    m_sbs.append(m_i)

                strip = spool.tile([128, nb, 64], F32)
                adv1 = spool.tile([128, nb], F32)
                adv2 = spool.tile([128, nb], F32)
                adrep1 = spool.tile([128, nlocp], F32)
                adrep2 = spool.tile([128, nlocp], F32)

                # ---- helpers --------------------------------------------------------
                def adrep_roundtrip(adv, adt_dram, adrep):
                    """adv [128, nb] (val for dst 128*b+p) -> adrep [128, nlocp]
                    (row-replicated) via DRAM roundtrip + ones-matmul broadcast."""
                    nc.sync.dma_start(out=adt_dram.ap().rearrange("b p -> p b"), in_=adv[:, :])
                    adrow = gpool.tile([1, nlocp], F32, tag="adrow")
                    nc.sync.dma_start(
                        out=adrow[:, :], in_=adt_dram.ap().flatten().unsqueeze(0)
                    )
                    for j0 in range(0, nlocp, 512):
                        w = min(512, nlocp - j0)
                        pt = pps.tile([128, 512], F32, tag="ps", name="ps")
                        nc.tensor.matmul(
                            pt[:, :w], ones1_sb[:, :], adrow[:, j0 : j0 + w],
                            start=True, stop=True,
                        )
                        nc.vector.tensor_copy(adrep[:, j0 : j0 + w], pt[:, :w])

                def strip_out(ccin):
                    if nb > 1:
                        nc.sync.dma_start(
                            out=ccin.ap()[0 : 128 * (nb - 1), :].rearrange(
                                "(b p) f -> p b f", p=128
                            ),
                            in_=strip[:, 0 : nb - 1, :],
                        )
                    nc.sync.dma_start(
                        out=ccin.ap()[128 * (nb - 1) : nloc, :],
                        in_=strip[0:tail, nb - 1, :],
                    )

                def allgather(ccin, ccout, haug, pad_idx):
                    nc.gpsimd.collective_compute(
                        "AllGather",
                        ALU.bypass,
                        replica_groups=groups,
                        ins=[ccin.ap().opt()],
                        outs=[ccout.ap().opt()],
                    )
                    nc.sync.dma_start(out=haug.ap()[0:N, :], in_=ccout.ap()[:, :])
                    nc.sync.dma_start(
                        out=haug.ap()[N : N + 1, :], in_=pads_d.ap()[pad_idx : pad_idx + 1, :]
                    )

                stopped = False

                def _dummy_out():
                    dz = gpool.tile([128, 512], F32, tag="dz")
                    nc.vector.memset(dz[:, :], 0.0)
                    nc.sync.dma_start(out=out_d.ap()[0:128, 0:512], in_=dz[:, :])

                # ---- phase B: layer-1 linear on own nodes --------------------------
                scr = gpool.tile([128, F1], F32, tag="scr")
                for b in range(nb):
                    ph = pps.tile([128, 512], F32, tag="ps", name="ps")[:, 0:F1]
                    nc.tensor.matmul(
                        ph[:, :], xt_sb[:, 128 * b : 128 * (b + 1)], w1_sb[:, :],
                        start=True, stop=True,
                    )
                    nc.vector.tensor_copy(strip[:, b, 0:F1], ph[:, :])
                    nc.vector.memset(strip[:, b, F1 : F1 + 1], 1.0)
                    nc.vector.memset(strip[:, b, F1 + 2 : 64], 0.0)
                    nc.vector.tensor_mul(scr[:, :], ph[:, :], asrc1_sb[:, :])
                    nc.vector.reduce_sum(
                        strip[:, b, F1 + 1 : F1 + 2], scr[:, :], axis=mybir.AxisListType.X
                    )
                    nc.vector.tensor_mul(scr[:, :], ph[:, :], adst1_sb[:, :])
                    nc.vector.reduce_sum(
                        adv1[:, b : b + 1], scr[:, :], axis=mybir.AxisListType.X
                    )
                if stop_after == "B0":
                    _dummy_out(); stopped = True
                if not stopped:
                    strip_out(ccin1)
                    allgather(ccin1, ccout1, haug1, 0)
                    adrep_roundtrip(adv1, adt1, adrep1)
                if stop_after == "B" and not stopped:
                    _dummy_out(); stopped = True

                # ---- edge layer ----------------------------------------------------
                EDGE_LVL = DEBUG_EDGE_LVL  # 3 = full edge phase (debug knob)

                def edge_layer(haug, adrep, fin, bias_sb, out_block):
                    """Aggregate one GAT layer for all own blocks.

                    haug rows: [h (fin) | 1 | as | pad]; for each block produces
                    z = relu(agg/s + b) [128, fin] and calls out_block(b, z_ap).
                    """
                    scol = fin  # ones column -> denominator
                    acol = fin + 1
                    rN = fin + 2  # matmul rhs width
                    for b in range(nb):
                        T = tpool.tile([128, K, 64], F32, tag="T")
                        # SWDGE ring caps a single gather around ~1-2k descriptors;
                        # split into 1024-idx chunks (16 c-columns each)
                        GCH = 1024
                        for q in range(SPB // GCH):
                            nc.gpsimd.dma_gather(
                                T[:, q * (GCH // 128) : (q + 1) * (GCH // 128), :],
                                haug.ap()[:, :],
                                elli_sb[
                                    :,
                                    b * ICPB + q * (GCH // 16) : b * ICPB + (q + 1) * (GCH // 16),
                                ],
                                GCH,
                                GCH,
                                64,
                            )
                        if EDGE_LVL < 1:
                            continue
                        adT = gpool.tile([128, K], F32, tag="adT")
                        for g in range(G):
                            nc.vector.tensor_copy(
                                adT[g * K : (g + 1) * K, :],
                                adrep[g * K : (g + 1) * K, 128 * b + g : 128 * (b + 1) : G],
                            )
                        ew = gpool.tile([128, K], F32, tag="ew")
                        nc.vector.tensor_add(ew[:, :], T[:, :, acol], adT[:, :])
                        # leaky_relu(e) = max(e, NEG*e), then exp on ACT
                        nc.vector.scalar_tensor_tensor(
                            ew[:, :], ew[:, :], NEG, ew[:, :], ALU.mult, ALU.max
                        )
                        nc.scalar.activation(ew[:, :], ew[:, :], AF.Exp)
                        # scatter w into the fixed M pattern:
                        # slot (p, t) -> dst D = G*t + p//K, offset t*128 + D
                        m_sb = m_sbs[b % 2]
                        mv = m_sb[:, :, :].rearrange("p a b -> p (a b)")
                        for g in range(G):
                            nc.vector.tensor_copy(
                                mv[g * K : (g + 1) * K, g : g + (K - 1) * (128 + G) + 1 : 128 + G],
                                ew[g * K : (g + 1) * K, :],
                            )
                        if EDGE_LVL < 2:
                            continue
                        agg = pagg.tile([128, 64], F32, tag="agg")
                        for t in range(K):
                            nc.tensor.matmul(
                                agg[:, 0:rN],
                                m_sb[:, t, :],
                                T[:, t, 0:rN],
                                start=(t == 0),
                                stop=(t == K - 1),
                            )
                        if EDGE_LVL < 3:
                            continue
                        rec = gpool.tile([128, 1], F32, tag="rec")
                        nc.vector.reciprocal(rec[:, :], agg[:, scol : scol + 1])
                        z = gpool.tile([128, 64], F32, tag="z")
                        nc.vector.tensor_scalar(
                            z[:, 0:fin], agg[:, 0:fin], rec[:, :], None, ALU.mult
                        )
                        nc.vector.tensor_add(z[:, 0:fin], z[:, 0:fin], bias_sb[:, :])
                        nc.scalar.activation(z[:, 0:fin], z[:, 0:fin], AF.Relu)
                        out_block(b, z)

                # ---- layer-1 consumer: h2 = z1 @ W2, rebuild strip -----------------
                def l1_out(b, z):
                    zt = pps.tile([128, 512], F32, tag="ps", name="ps")[0:F1, 0:128]
                    nc.tensor.transpose(zt[:, :], z[:, 0:F1], eye_sb[:, :])
                    ztsb = gpool.tile([F1, 128], F32, tag="ztsb")
                    nc.vector.tensor_copy(ztsb[:, :], zt[:, :])
                    ph2 = pps.tile([128, 512], F32, tag="ps", name="ps")[:, 0:F2]
                    nc.tensor.matmul(ph2[:, :], ztsb[:, :], w2_sb[:, :], start=True, stop=True)
                    nc.vector.tensor_copy(strip[:, b, 0:F2], ph2[:, :])
                    nc.vector.memset(strip[:, b, F2 : F2 + 1], 1.0)
                    nc.vector.memset(strip[:, b, F2 + 2 : 64], 0.0)
                    scr2 = gpool.tile([128, F2], F32, tag="scr2")
                    nc.vector.tensor_mul(scr2[:, :], ph2[:, :], asrc2_sb[:, :])
                    nc.vector.reduce_sum(
                        strip[:, b, F2 + 1 : F2 + 2], scr2[:, :], axis=mybir.AxisListType.X
                    )
                    nc.vector.tensor_mul(scr2[:, :], ph2[:, :], adst2_sb[:, :])
                    nc.vector.reduce_sum(
                        adv2[:, b : b + 1], scr2[:, :], axis=mybir.AxisListType.X
                    )

                if not stopped:
                    edge_layer(haug1, adrep1, F1, b1_sb, l1_out)
                if stop_after == "C" and not stopped:
                    _dummy_out(); stopped = True
                if not stopped:
                    strip_out(ccin2)
                    allgather(ccin2, ccout2, haug2, 1)
                    adrep_roundtrip(adv2, adt2, adrep2)

                # ---- layer-2 consumer: transpose z2 into zt_own --------------------
                def l2_out(b, z):
                    zt = pps.tile([128, 512], F32, tag="ps", name="ps")[0:F2, 0:128]
                    nc.tensor.transpose(zt[:, :], z[:, 0:F2], eye_sb[:, :])
                    nc.vector.tensor_copy(zt_own[:, 128 * b : 128 * (b + 1)], zt[:, :])

                if not stopped:
                    edge_layer(haug2, adrep2, F2, b2_sb, l2_out)
                if stop_after == "D" and not stopped:
                    _dummy_out(); stopped = True

                if not stopped:
                    # share z (transposed) with all cores
                    nc.sync.dma_start(out=ztin.ap()[:, :], in_=zt_own[:, 0:nloc].bitcast(F32))
                    nc.gpsimd.collective_compute(
                        "AllGather",
                        ALU.bypass,
                        replica_groups=groups,
                        ins=[ztin.ap().opt()],
                        outs=[ztcc.ap().opt()],
                    )
                else:
                    nc.vector.memset(zt_own[:, :].bitcast(F32), 0.0)
                    nc.sync.dma_start(out=ztcc.ap()[0:F2, :], in_=zt_own[:, 0:nloc].bitcast(F32))

            # ---- decoder (separate pool scope so GAT SBUF is reusable) ------------
            with (
                tc.tile_pool(name="dec", bufs=1) as dpool,
                tc.tile_pool(name="dec_rows", bufs=2) as rpool,
                tc.tile_pool(name="psum_dec", bufs=4, space="PSUM") as pdec,
            ):
                if stopped:
                    P_eff = 0
                    nb_eff = 0
                else:
                    P_eff = P
                    nb_eff = nb
                ztf = dpool.tile([F2, N], F32)
                for r in range(P_eff):
                    nc.sync.dma_start(
                        out=ztf[:, r * nloc : (r + 1) * nloc],
                        in_=ztcc.ap()[r * F2 : (r + 1) * F2, :],
                    )
                ztfr = dpool.tile([F2, N], F32R)
                if not stopped:
                    nc.vector.tensor_copy(ztfr[:, :], ztf[:, :])
                for b in range(nb_eff):
                    rows = 128 if b < nb - 1 else tail
                    srow = rpool.tile([128, N], F32, tag="srow")
                    for ch in range(NCH):
                        j0 = ch * DCH
                        pd = pdec.tile([128, DCH], F32, tag="pd")
                        nc.tensor.matmul(
                            pd[:, :],
                            zt_own[:, 128 * b : 128 * (b + 1)],
                            ztfr[:, j0 : j0 + DCH],
                            start=True,
                            stop=True,
                        )
                        nc.scalar.activation(srow[:, j0 : j0 + DCH], pd[:, :], AF.Sigmoid)
                    nc.sync.dma_start(
                        out=out_d.ap()[128 * b : 128 * b + rows, :], in_=srow[0:rows, :]
                    )

        for _rep in range(repeat):
            _pipeline()
            if stop_after is not None and repeat > 1:
                tc.strict_bb_all_engine_barrier()

    nc.compile()
    return nc


# --------------------------------------------------------------------------
# entry point
# --------------------------------------------------------------------------

_CACHE = {}
TRACE = False
LAST_RESULT = None


def _get_program(key="full"):
    if key not in _CACHE:
        _CACHE[key] = build_program(derive(FULL_CFG))
    return _CACHE[key]


def kernel(x, edge_index, W1, a_src1, a_dst1, b1, W2, a_src2, a_dst2, b2, **_):
    base = dict(FULL_CFG)
    # ELL width: 64 covers the reference graph (max in-degree 55); fall back
    # to 128 for denser graphs.
    ei = np.asarray(edge_index)
    deg = np.bincount(
        np.concatenate([ei[1].astype(np.int64), np.arange(base["N"])]),
        minlength=base["N"],
    )
    if deg.max() > 64:
        base["K"] = 128
    cfg = derive(base)
    maps = make_inputs(
        x, edge_index, W1, a_src1, a_dst1, b1, W2, a_src2, a_dst2, b2, cfg
    )
    key = ("full", base["K"])
    if key not in _CACHE:
        _CACHE[key] = build_program(cfg)
    nc = _CACHE[key]
    global LAST_RESULT
    res = run_bass_kernel_spmd(nc, maps, list(range(cfg["P"])), trace=TRACE)
    LAST_RESULT = res
    out = np.concatenate([res.results[i]["out"] for i in range(cfg["P"])], axis=0)
    return out.astype(np.float32)



# revision 13
# speedup vs baseline: 3.9618x; 3.9618x over previous
"""Trainium2 Bass kernel for nn_Decoder_68152541053662.

2-layer GAT (heads=1, self-loops) + sigmoid inner-product decoder.
  N=12000 nodes, E=384000 edges (+N self loops), feats 40 -> 50 -> 40,
  output sigmoid(z @ z.T)  [12000, 12000] f32.

v2 design:
  * Nodes row-partitioned across 8 cores (1500 per core) with a per-core
    DESCENDING degree sort, so each 128-dst block b can use its own ELL
    width K_b = cross-core max in-degree of block b (rounded up to 8)
    instead of a global K=64.  All indices are host-built.
  * Feature tables are [12288, 128] bf16 in DRAM; row = [h | 1 | a_src.h
    | a_dst.h | garbage] (gather wants 256-byte rows).  Table row of
    (core, pos) = 4096*(pos//512) + 512*core + pos%512, which makes each
    AllGather *piece* (4 blocks of pos-rows) land contiguously.
  * Layer-1 table is computed fully locally by every core (x is
    replicated) -- no collective at all.  Layer-2 / decoder tables are
    built with 3 piece-AllGathers (bf16) issued as soon as their 4
    source blocks are done, overlapping the remaining edge compute.
  * Edge aggregation per block: dma_gather T[p, c, :] = c-th in-neighbor
    of dst p (self-loop first, so T[:, 0, dcol] is the dst's own
    a_dst.h), then ew = exp(leaky_relu(as + ad)) and a DVE
    scalar_tensor_tensor chain accumulates agg += ew[:, c] * T[:, c, :].
    The ones-column doubles as the softmax denominator.  No selection
    matrices, no PSUM, no a_dst replication roundtrip.
  * Decoder exploits symmetry of sigmoid(z z^T): global 128-row blocks
    are owned block-cyclically (core c owns g = 8l + c) and block l only
    computes columns >= 500*((1024*l)//500).  The host mirrors the lower
    triangle.  Output is bf16 (tolerance 2e-2, bf16 adds ~2e-3), halving
    the dominant DMA.  Both matmul operands are rebuilt in TRUE node
    order from the allgathered z table with dma_gather + PE transposes.
"""

import numpy as np

try:
    import concourse.bass as bass
except ImportError:  # pragma: no cover
    import sys

    sys.path.insert(0, "/opt/trn_rl_repo")
    import concourse.bass as bass

import concourse.bacc as bacc
import concourse.tile as tile
from concourse import mybir
from concourse.bass_utils import run_bass_kernel_spmd

AF = mybir.ActivationFunctionType
ALU = mybir.AluOpType
F32 = mybir.dt.float32
F32R = mybir.dt.float32r
BF16 = mybir.dt.bfloat16
I16 = mybir.dt.int16

FULL_CFG = dict(N=12000, P=8, FIN=40, F1=50, F2=40, NEG=0.2, DEC_CHUNK=500)
PAD_AS = -100.0  # "as" value of pad rows -> w ~ exp(-20) ~ 0
GCH = 1024  # gather chunk (SWDGE ring cap)
NQ = 1  # SWDGE queues
NPIECE = 3  # allgather pieces per table


def derive(cfg):
    d = dict(cfg)
    N, P = cfg["N"], cfg["P"]
    d["NLOC"] = N // P  # real nodes per core (1500)
    d["NB"] = (d["NLOC"] + 127) // 128  # pos blocks per core (12)
    d["NLOCP"] = d["NB"] * 128  # padded pos rows (1536)
    d["PROWS"] = d["NLOCP"] // NPIECE  # pos rows per piece (512)
    d["TPIECE"] = d["PROWS"] * P  # table rows per piece (4096)
    d["NTAB"] = d["TPIECE"] * NPIECE  # table rows (12288)
    d["TBLK"] = d["NTAB"] // 128  # table blocks (96)
    d["NBLK"] = (N + 127) // 128  # global decoder row blocks (94)
    d["DNB"] = (d["NBLK"] + P - 1) // P  # decode blocks per core (12)
    d["NBG"] = d["DNB"] * P  # padded global block count (96)
    d["NCH"] = N // cfg["DEC_CHUNK"]
    assert N % cfg["DEC_CHUNK"] == 0 and cfg["DEC_CHUNK"] <= 512
    assert N % P == 0 and d["NLOCP"] % NPIECE == 0 and d["PROWS"] % 128 == 0
    return d


def table_row(core, pos, cfg):
    """DRAM table row for a node at sorted position `pos` on `core`."""
    pr = cfg["PROWS"]
    return cfg["TPIECE"] * (pos // pr) + pr * core + pos % pr


# --------------------------------------------------------------------------
# host-side preprocessing
# --------------------------------------------------------------------------


def wrap_idx(flat):
    """flat idx list (len % 16 == 0) -> [128, len//16] int16 wrapped layout."""
    flat = np.asarray(flat, dtype=np.int16)
    cols = flat.reshape(-1, 16).T  # [16, len//16]
    return np.tile(cols, (8, 1))


def analyze_graph(edge_index, cfg):
    """Degree-sort nodes per core, pick the per-block K schedule, and build
    per-core ELL index tables (flat order J = c*128 + p, self-loop first)."""
    c = cfg
    N, P = c["N"], c["P"]
    nloc, nb = c["NLOC"], c["NB"]
    src = np.asarray(edge_index[0], dtype=np.int64)
    dst = np.asarray(edge_index[1], dtype=np.int64)
    deg = np.bincount(dst, minlength=N) + 1  # +1 self loop

    perms = []  # perms[core][pos] = original local node id
    rows_of = np.zeros(N, dtype=np.int64)  # node -> gather-table row
    for core in range(P):
        d_loc = deg[core * nloc : (core + 1) * nloc]
        perm = np.argsort(d_loc, kind="stable")
        perms.append(perm)
        inv = np.empty(nloc, dtype=np.int64)
        inv[perm] = np.arange(nloc)
        rows_of[core * nloc : (core + 1) * nloc] = table_row(core, inv, c)

    kmat = np.zeros((P, nb), dtype=np.int64)
    for core in range(P):
        d_sorted = deg[core * nloc : (core + 1) * nloc][perms[core]]
        for b in range(nb):
            seg = d_sorted[128 * b : min(128 * (b + 1), nloc)]
            kmat[core, b] = seg.max() if len(seg) else 0
    ks = tuple(int(max(4, kmat[:, b].max())) for b in range(nb))

    order = np.argsort(dst, kind="stable")
    src_s = src[order]
    deg_e = deg - 1  # edge-only degree
    starts = np.concatenate([[0], np.cumsum(deg_e)])

    pad_row = table_row(0, nloc, c)  # core 0's first pad row
    ells = []
    for core in range(P):
        perm = perms[core]
        flat_parts = []
        for b in range(nb):
            kb = ks[b]
            slots = np.full((128, kb), pad_row, dtype=np.int64)  # [p, c]
            for p in range(min(128, nloc - 128 * b)):
                n = core * nloc + perm[128 * b + p]
                slots[p, 0] = rows_of[n]  # self loop first
                srcs = src_s[starts[n] : starts[n + 1]]
                slots[p, 1 : 1 + len(srcs)] = rows_of[srcs]
            flat_parts.append(slots.T.reshape(-1))  # J = c*128 + p
        ells.append(wrap_idx(np.concatenate(flat_parts)))
    return perms, rows_of, ks, ells


def make_inputs(x, edge_index, W1, a_src1, a_dst1, b1, W2, a_src2, a_dst2, b2, cfg):
    import ml_dtypes

    c = cfg
    P, FIN, F1, F2 = c["P"], c["FIN"], c["F1"], c["F2"]
    N, nloc, ntab = c["N"], c["NLOC"], c["NTAB"]
    dnb, nbg, nblk = c["DNB"], c["NBG"], c["NBLK"]
    x = np.asarray(x, dtype=np.float32)
    perms, rows_of, ks, ells = analyze_graph(edge_index, c)

    def rep(v, f):
        r = np.zeros((128, f), dtype=np.float32)
        r[:] = np.asarray(v, dtype=np.float32)[None, :]
        return r

    def aug_w(W, a_s, a_d):
        W = np.asarray(W, dtype=np.float32)
        return np.concatenate(
            [
                W,
                (W @ np.asarray(a_s, dtype=np.float32))[:, None],
                (W @ np.asarray(a_d, dtype=np.float32))[:, None],
            ],
            axis=1,
        )

    # x replicated in table order
    xt = np.zeros((FIN, ntab), dtype=np.float32)
    for core in range(P):
        xt[:, rows_of[core * nloc : (core + 1) * nloc]] = (
            x[core * nloc : (core + 1) * nloc].T
        )
    xt = xt.astype(ml_dtypes.bfloat16)

    npad = c["NLOCP"] - nloc  # 36 pad rows per core
    def padrows(fin):
        r = np.zeros((npad, 64), dtype=np.float32)
        r[:, fin] = 1.0
        r[:, fin + 1] = PAD_AS
        return r.astype(ml_dtypes.bfloat16)

    rhs_flat = np.zeros(nbg * 128, dtype=np.int64)
    rhs_flat[:N] = rows_of
    common = {
        "padr1": np.tile(padrows(F1)[None, :, :], (P, 1, 1)),
        "padr2": padrows(F2),
        "xt": xt,
        "w1a": aug_w(W1, a_src1, a_dst1).astype(ml_dtypes.bfloat16),
        "w2a": aug_w(W2, a_src2, a_dst2),
        "b1r": rep(b1, F1),
        "b2r": rep(b2, F2),
        "eye": np.eye(128, dtype=np.float32),
        "rhsi": wrap_idx(rhs_flat),
    }
    maps = []
    for core in range(P):
        lhs_flat = np.zeros(dnb * 128, dtype=np.int64)
        for l in range(dnb):
            g = P * l + core
            if g >= nblk:
                continue
            n0, n1 = 128 * g, min(128 * g + 128, N)
            lhs_flat[128 * l : 128 * l + (n1 - n0)] = rows_of[n0:n1]
        m = dict(common)
        m["elli"] = ells[core]
        m["deci"] = wrap_idx(lhs_flat)
        maps.append(m)
    return maps, ks


# --------------------------------------------------------------------------
# device program
# --------------------------------------------------------------------------


def build_program(cfg, ks, stop_after=None, repeat=1):
    c = cfg
    N, P, FIN, F1, F2, NEG = c["N"], c["P"], c["FIN"], c["F1"], c["F2"], c["NEG"]
    nloc, nb, ntab, tblk = c["NLOC"], c["NB"], c["NTAB"], c["TBLK"]
    prows, tpiece = c["PROWS"], c["TPIECE"]
    dnb, nbg = c["DNB"], c["NBG"]
    DCH, NCH = c["DEC_CHUNK"], c["NCH"]
    groups = [list(range(P))]
    icols = [128 * kb // 16 for kb in ks]
    ioffs = np.concatenate([[0], np.cumsum(icols)]).tolist()
    kmax = max(ks)
    bpp = nb // NPIECE  # pos blocks per piece (4)

    nc = bacc.Bacc(
        "TRN2",
        target_bir_lowering=False,
        debug=False,
        num_devices=P,
        num_swdge_queues=NQ,
    )

    # I/O
    xt_d = nc.dram_tensor("xt", [FIN, ntab], BF16, kind="ExternalInput")
    w1a_d = nc.dram_tensor("w1a", [FIN, F1 + 2], BF16, kind="ExternalInput")
    w2a_d = nc.dram_tensor("w2a", [F1, F2 + 2], F32, kind="ExternalInput")
    b1r_d = nc.dram_tensor("b1r", [128, F1], F32, kind="ExternalInput")
    b2r_d = nc.dram_tensor("b2r", [128, F2], F32, kind="ExternalInput")
    eye_d = nc.dram_tensor("eye", [128, 128], F32, kind="ExternalInput")
    elli_d = nc.dram_tensor("elli", [128, ioffs[-1]], I16, kind="ExternalInput")
    deci_d = nc.dram_tensor("deci", [128, dnb * 128 // 16], I16, kind="ExternalInput")
    rhsi_d = nc.dram_tensor("rhsi", [128, nbg * 128 // 16], I16, kind="ExternalInput")
    npad = c["NLOCP"] - nloc
    padr1_d = nc.dram_tensor("padr1", [P, npad, 64], BF16, kind="ExternalInput")
    padr2_d = nc.dram_tensor("padr2", [npad, 64], BF16, kind="ExternalInput")
    out_d = nc.dram_tensor("out", [dnb * 128, N], BF16, kind="ExternalOutput")

    # internal DRAM
    tabs = [nc.dram_tensor(f"tab{i}", [ntab, 128], BF16) for i in range(3)]
    ccp = [
        [nc.dram_tensor(f"cc{i}_{p}", [prows, 64], BF16) for p in range(NPIECE)]
        for i in range(2)
    ]
    tpp = [
        [
            nc.dram_tensor(f"tp{i}_{p}", [tpiece, 64], BF16, addr_space="Shared")
            for p in range(NPIECE)
        ]
        for i in range(2)
    ]

    with tile.TileContext(nc) as tc:

        def _pipeline():
            with (
                tc.tile_pool(name="const", bufs=1) as cpool,
                tc.tile_pool(name="strips", bufs=1) as spool,
                tc.tile_pool(name="gat_small", bufs=3) as gpool,
                tc.tile_pool(name="gather", bufs=2) as tpool,
                tc.tile_pool(name="psum_small", bufs=3, space="PSUM") as pps,
            ):
                # ---- constant loads ---------------------------------------
                xt_sb = cpool.tile([FIN, ntab], BF16)
                nc.sync.dma_start(out=xt_sb[:, :], in_=xt_d[:, :])
                w1a_sb = cpool.tile([FIN, F1 + 2], BF16)
                nc.sync.dma_start(out=w1a_sb[:, :], in_=w1a_d[:, :])
                w2a_sb = cpool.tile([F1, F2 + 2], F32)
                nc.sync.dma_start(out=w2a_sb[:, :], in_=w2a_d[:, :])
                b1_sb = cpool.tile([128, F1], F32)
                nc.sync.dma_start(out=b1_sb[:, :], in_=b1r_d[:, :])
                b2_sb = cpool.tile([128, F2], F32)
                nc.sync.dma_start(out=b2_sb[:, :], in_=b2r_d[:, :])
                eye_sb = cpool.tile([128, 128], F32)
                nc.sync.dma_start(out=eye_sb[:, :], in_=eye_d[:, :])
                elli_sb = cpool.tile([128, ioffs[-1]], I16)
                nc.sync.dma_start(out=elli_sb[:, :], in_=elli_d[:, :])

                bstrip = spool.tile([128, tblk, 64], BF16)  # table-space strip
                zs = spool.tile([128, nb, 64], BF16)  # pos-space strip

                stopped = False

                def _dummy_out():
                    dz = gpool.tile([128, 512], BF16, tag="dz")
                    nc.vector.memset(dz[:, :], 0.0)
                    nc.sync.dma_start(out=out_d.ap()[0:128, 0:512], in_=dz[:, :])

                # ---- phase B: full layer-1 table, locally on every core ---
                nc.vector.memset(bstrip[:, :, :], 0.0)
                nc.vector.memset(zs[:, :, :], 0.0)
                nc.vector.memset(bstrip[:, :, F1 : F1 + 1], 1.0)
                for i in range(tblk // 2):
                    ph = pps.tile([128, 2, F1 + 2], F32, tag="ps", name="ps")
                    for j in range(2):
                        t = 2 * i + j
                        nc.tensor.matmul(
                            ph[:, j, :], xt_sb[:, 128 * t : 128 * (t + 1)],
                            w1a_sb[:, :], start=True, stop=True,
                        )
                    nc.vector.tensor_copy(
                        bstrip[:, 2 * i : 2 * i + 2, 0:F1], ph[:, :, 0:F1]
                    )
                    nc.vector.tensor_copy(
                        bstrip[:, 2 * i : 2 * i + 2, F1 + 1 : F1 + 3],
                        ph[:, :, F1 : F1 + 2],
                    )
                nc.sync.dma_start(
                    out=tabs[0].ap()[:, 0:64].rearrange("(t p) f -> p t f", p=128),
                    in_=bstrip[:, :, :],
                )
                # pad rows (pos >= nloc on every core): h=0, ones, as=PAD_AS
                nc.sync.dma_start(
                    out=tabs[0]
                    .ap()[2 * tpiece :, 0:64]
                    .rearrange("(c r) f -> c r f", r=prows)[
                        :, nloc % prows : nloc % prows + npad, :
                    ],
                    in_=padr1_d[:, :, :],
                )
                if stop_after == "B0":
                    _dummy_out(); stopped = True

                # ---- piece allgather machinery ----------------------------
                def piece_ag(li, p):
                    nc.sync.dma_start(
                        out=ccp[li][p].ap().rearrange("(b q) f -> q b f", q=128),
                        in_=zs[:, bpp * p : bpp * (p + 1), :],
                    )
                    if li == 0 and p == NPIECE - 1:
                        nc.sync.dma_start(
                            out=ccp[li][p].ap()[nloc % prows : nloc % prows + npad, :],
                            in_=padr2_d[:, :],
                        )
                    nc.gpsimd.collective_compute(
                        "AllGather",
                        ALU.bypass,
                        replica_groups=groups,
                        ins=[ccp[li][p].ap().opt()],
                        outs=[tpp[li][p].ap().opt()],
                    )
                    nc.sync.dma_start(
                        out=tabs[li + 1].ap()[tpiece * p : tpiece * (p + 1), 0:64],
                        in_=tpp[li][p].ap()[:, :],
                    )

                # ---- edge layer -------------------------------------------
                def edge_layer(tab, fin, bias_sb, out_block, li):
                    scol, acol, dcol = fin, fin + 1, fin + 2
                    rN = fin + 1  # h cols + ones col
                    for b in range(nb):
                        kb = ks[b]
                        T = tpool.tile([128, kmax, 128], BF16, tag="T")
                        for q0 in range(0, 128 * kb, GCH):
                            n = min(GCH, 128 * kb - q0)
                            nc.gpsimd.dma_gather(
                                T[:, q0 // 128 : (q0 + n) // 128, :],
                                tab.ap()[:, :],
                                elli_sb[
                                    :, ioffs[b] + q0 // 16 : ioffs[b] + (q0 + n) // 16
                                ],
                                n,
                                n,
                                128,
                                queue_num=0,
                            )
                        adc = gpool.tile([128, 1], F32, tag="adc")
                        nc.vector.tensor_copy(adc[:, :], T[:, 0, dcol : dcol + 1])
                        ew = gpool.tile([128, kmax], F32, tag="ew")
                        nc.vector.tensor_scalar(
                            ew[:, 0:kb], T[:, 0:kb, acol], adc[:, :], None, ALU.add
                        )
                        nc.vector.scalar_tensor_tensor(
                            ew[:, 0:kb], ew[:, 0:kb], NEG, ew[:, 0:kb], ALU.mult, ALU.max
                        )
                        nc.scalar.activation(ew[:, 0:kb], ew[:, 0:kb], AF.Exp)
                        agg = gpool.tile([128, 64], F32, tag="agg")
                        nc.vector.tensor_scalar(
                            agg[:, 0:rN], T[:, 0, 0:rN], ew[:, 0:1], None, ALU.mult
                        )
                        for q in range(1, kb):
                            nc.vector.scalar_tensor_tensor(
                                agg[:, 0:rN],
                                T[:, q, 0:rN],
                                ew[:, q : q + 1],
                                agg[:, 0:rN],
                                ALU.mult,
                                ALU.add,
                            )
                        rec = gpool.tile([128, 1], F32, tag="rec")
                        nc.vector.reciprocal(rec[:, :], agg[:, scol : scol + 1])
                        z = gpool.tile([128, 64], F32, tag="z")
                        nc.vector.tensor_scalar(
                            z[:, 0:fin], agg[:, 0:fin], rec[:, :], None, ALU.mult
                        )
                        nc.vector.tensor_add(z[:, 0:fin], z[:, 0:fin], bias_sb[:, :])
                        nc.scalar.activation(z[:, 0:fin], z[:, 0:fin], AF.Relu)
                        out_block(b, z)
                        if li is not None and (b + 1) % bpp == 0:
                            piece_ag(li, b // bpp)

                # ---- layer-1 consumer: h2 table rows into zs --------------
                def l1_out(b, z):
                    zt = pps.tile([128, 512], F32, tag="ps", name="ps")[0:F1, 0:128]
                    nc.tensor.transpose(zt[:, :], z[:, 0:F1], eye_sb[:, :])
                    ztsb = gpool.tile([F1, 128], F32, tag="ztsb")
                    nc.vector.tensor_copy(ztsb[:, :], zt[:, :])
                    ph2 = pps.tile([128, 512], F32, tag="ps", name="ps")[:, 0 : F2 + 2]
                    nc.tensor.matmul(
                        ph2[:, :], ztsb[:, :], w2a_sb[:, :], start=True, stop=True
                    )
                    nc.vector.tensor_copy(zs[:, b, 0:F2], ph2[:, 0:F2])
                    nc.vector.tensor_copy(zs[:, b, F2 + 1 : F2 + 3], ph2[:, F2 : F2 + 2])

                if not stopped:
                    nc.vector.memset(zs[:, :, F2 : F2 + 1], 1.0)
                    edge_layer(tabs[0], F1, b1_sb, l1_out, li=0)
                if stop_after == "C" and not stopped:
                    _dummy_out(); stopped = True

                # ---- layer-2 consumer: z rows into zs ---------------------
                def l2_out(b, z):
                    nc.vector.tensor_copy(zs[:, b, 0:F2], z[:, 0:F2])

                if not stopped:
                    edge_layer(tabs[1], F2, b2_sb, l2_out, li=1)
                if stop_after == "D" and not stopped:
                    _dummy_out(); stopped = True

                if stopped:
                    return False
            return True

        def _decoder(ok):
            with (
                tc.tile_pool(name="dec", bufs=1) as dpool,
                tc.tile_pool(name="dec_rows", bufs=2) as rpool,
                tc.tile_pool(name="psum_dec", bufs=2, space="PSUM") as pdec,
            ):
                if not ok:
                    return
                eye2 = dpool.tile([128, 128], F32)
                nc.sync.dma_start(out=eye2[:, :], in_=eye_d[:, :])
                eyeb = dpool.tile([128, 128], BF16)
                nc.vector.tensor_copy(eyeb[:, :], eye2[:, :])
                deci_sb = dpool.tile([128, dnb * 128 // 16], I16)
                nc.sync.dma_start(out=deci_sb[:, :], in_=deci_d[:, :])
                rhsi_sb = dpool.tile([128, nbg * 128 // 16], I16)
                nc.sync.dma_start(out=rhsi_sb[:, :], in_=rhsi_d[:, :])

                # lhsT: gather owned blocks in true order, transpose
                zl = dpool.tile([128, dnb, 128], BF16)
                nq = dnb * 128 // GCH + (1 if (dnb * 128) % GCH else 0)
                for q in range(nq):
                    n0, n1 = q * GCH, min((q + 1) * GCH, dnb * 128)
                    nc.gpsimd.dma_gather(
                        zl[:, n0 // 128 : n1 // 128, :],
                        tabs[2].ap()[:, :],
                        deci_sb[:, n0 // 16 : n1 // 16],
                        n1 - n0,
                        n1 - n0,
                        128,
                        queue_num=0,
                    )
                ztl = dpool.tile([F2, dnb * 128], BF16)
                for l in range(dnb):
                    pt = pdec.tile([128, 512], BF16, tag="pt", name="pt")[0:F2, 0:128]
                    nc.tensor.transpose(pt[:, :], zl[:, l, 0:F2], eyeb[:, :])
                    nc.vector.tensor_copy(ztl[:, 128 * l : 128 * (l + 1)], pt[:, :])

                # rhs: gather the full table in true order, transpose
                zr = dpool.tile([128, nbg, 128], BF16)
                for q in range(nbg * 128 // GCH):
                    nc.gpsimd.dma_gather(
                        zr[:, q * (GCH // 128) : (q + 1) * (GCH // 128), :],
                        tabs[2].ap()[:, :],
                        rhsi_sb[:, q * (GCH // 16) : (q + 1) * (GCH // 16)],
                        GCH,
                        GCH,
                        128,
                        queue_num=0,
                    )
                ztfr = dpool.tile([F2, nbg * 128], BF16)
                for t in range(nbg):
                    if 128 * t >= N:
                        nc.vector.memset(ztfr[:, 128 * t : 128 * (t + 1)], 0.0)
                        continue
                    pt = pdec.tile([128, 512], BF16, tag="pt", name="pt")[0:F2, 0:128]
                    nc.tensor.transpose(pt[:, :], zr[:, t, 0:F2], eyeb[:, :])
                    nc.vector.tensor_copy(ztfr[:, 128 * t : 128 * (t + 1)], pt[:, :])

                for l in range(dnb):
                    c0 = (1024 * l) // DCH
                    width = (NCH - c0) * DCH
                    srow = rpool.tile([128, N], BF16, tag="srow")
                    for chp in range((NCH - c0) // 2):
                        j0 = (c0 + 2 * chp) * DCH
                        pd = pdec.tile([128, 2, 512], F32, tag="pd", name="pd")
                        for j in range(2):
                            nc.tensor.matmul(
                                pd[:, j, 0:DCH],
                                ztl[:, 128 * l : 128 * (l + 1)],
                                ztfr[:, j0 + j * DCH : j0 + (j + 1) * DCH],
                                start=True,
                                stop=True,
                            )
                        o0 = j0 - c0 * DCH
                        nc.scalar.activation(
                            srow[:, o0 : o0 + 2 * DCH], pd[:, :, 0:DCH], AF.Sigmoid
                        )
                    nc.sync.dma_start(
                        out=out_d.ap()[128 * l : 128 * (l + 1), c0 * DCH : N],
                        in_=srow[:, 0:width],
                    )

        for _rep in range(repeat):
            ok = _pipeline()
            _decoder(ok)
            if repeat > 1:
                tc.strict_bb_all_engine_barrier()

    nc.compile()
    return nc


# --------------------------------------------------------------------------
# entry point
# --------------------------------------------------------------------------

_CACHE = {}
TRACE = False
LAST_RESULT = None


def kernel(x, edge_index, W1, a_src1, a_dst1, b1, W2, a_src2, a_dst2, b2, **_):
    cfg = derive(FULL_CFG)
    maps, ks = make_inputs(
        x, edge_index, W1, a_src1, a_dst1, b1, W2, a_src2, a_dst2, b2, cfg
    )
    key = ("full", ks)
    if key not in _CACHE:
        _CACHE[key] = build_program(cfg, ks)
    nc = _CACHE[key]
    global LAST_RESULT
    res = run_bass_kernel_spmd(nc, maps, list(range(cfg["P"])), trace=TRACE)
    LAST_RESULT = res

    N, P, dnb, nblk = cfg["N"], cfg["P"], cfg["DNB"], cfg["NBLK"]
    DCH = cfg["DEC_CHUNK"]
    full = np.empty((N, N), dtype=np.float32)
    for core in range(P):
        o = np.asarray(res.results[core]["out"])
        for l in range(dnb):
            g = P * l + core
            if g >= nblk:
                continue
            r0, r1 = 128 * g, min(128 * g + 128, N)
            cs = DCH * ((1024 * l) // DCH)
            full[r0:r1, cs:] = o[128 * l : 128 * l + (r1 - r0), cs:].astype(np.float32)
    for g in range(nblk):
        r0, r1 = 128 * g, min(128 * g + 128, N)
        cs = DCH * ((1024 * (g // P)) // DCH)
        if cs:
            full[r0:r1, 0:cs] = full[0:cs, r0:r1].T
    return full
